# revision 54
# baseline (speedup 1.0000x reference)
"""ChannelAttention Trainium2 Bass kernel (v3).

Data-parallel over batch: 8 batches -> 8 NeuronCores, zero communication.

Key algebra: q,k are never materialized.  With G = x^T x  [C, C]:
  gram_qk_h = Wq_h^T G Wk_h          (attention logits numerator)
  sumsq_q   = diag(Wq_h^T G Wq_h)    (token-dim L2 norms of q)
  sumsq_k   = diag(Wk_h^T G Wk_h)
Pass 1 accumulates G (upper triangle) from token chunks streamed over
BOTH HWDGE rings (even chunks sync, odd chunks scalar); weights and the
x^T prefetch queue behind them.  Finalize (fp16 operands, fp32 PSUM):
  phase K (rows descending, so row 5 needs no mirrored G blocks):
      A2k = G @ Wk, grams Wq_h^T A2k_h (qk) and Wk_h^T A2k_h (kk)
  phase Q: A2q = G @ Wq, grams Wq_h^T A2q_h (qq); the s_k/softmax-scale
      DVE chain runs concurrently with phase Q's PE work
  softmax, T1_h = attn_h^T Wproj_h, Wbig = sum_h Wv_h @ T1_h   [C, C]
Pass 2: y^T = Wbig^T @ x^T + b from SBUF-resident x^T; output rides
both rings.  Host transposes the output.

Gram lhsT slices are padded to 128 columns (extra output partitions are
never read) so LDWEIGHTS gets fast-weight-load.  fp16 everywhere: same
PE speed as bf16, 8x finer mantissa; |x|<6, |G|<5k, |A2|<1k << 65504.
"""

import sys

if "/opt/trn_rl_repo" not in sys.path:
    sys.path.insert(0, "/opt/trn_rl_repo")

import numpy as np

N, C, H, HD = 4096, 768, 8, 96
C2 = 2 * C
NC3 = 3 * C
EPS = 1e-12
P = 128
CB = C // P            # 6 channel blocks
NCH2 = N // 256        # 16 double token chunks
W2PAD = C2 + 32        # fp16 w2 padded so 128-wide lhsT slices stay in bounds

# upper-triangle block packing: block (r, c), r <= c, index b -> bank b//4,
# column offset (b%4)*128 inside PSUM tiles of [128, 512]
_STARTS = [0, 6, 11, 15, 18, 20]
# per row: list of (bank, offset, c0, ncols) matmul runs covering cols c0..
_G_RUNS = {
    0: [(0, 0, 0, 512), (1, 0, 4, 256)],
    1: [(1, 256, 1, 256), (2, 0, 3, 384)],
    2: [(2, 384, 2, 128), (3, 0, 3, 384)],
    3: [(3, 384, 3, 128), (4, 0, 4, 256)],
    4: [(4, 256, 4, 256)],
    5: [(5, 0, 5, 128)],
}

_CACHE = {}


def _blk(b):
    return b // 4, (b % 4) * P


def _build(dbg=False):
    import concourse.bacc as bacc
    import concourse.tile as tile
    import concourse.mybir as mybir
    from concourse.masks import make_identity
    from contextlib import ExitStack

    F32 = mybir.dt.float32
    F16 = mybir.dt.float16

    nc = bacc.Bacc("TRN2", target_bir_lowering=False, debug=False, num_devices=8)
    x = nc.dram_tensor("x", [N, C], F16, kind="ExternalInput")
    xt = nc.dram_tensor("xt", [C, N], F16, kind="ExternalInput")
    w2 = nc.dram_tensor("w2", [C, C2], F16, kind="ExternalInput")
    wvt = nc.dram_tensor("wvt", [HD, H, C], F16, kind="ExternalInput")
    wpe = nc.dram_tensor("wpe", [HD, H, C], F16, kind="ExternalInput")
    temp = nc.dram_tensor("temp", [H], F32, kind="ExternalInput")
    biasE = nc.dram_tensor("biasE", [P, CB], F32, kind="ExternalInput")
    yt = nc.dram_tensor("yt", [C, N], F16, kind="ExternalOutput")

    with tile.TileContext(nc) as tc, ExitStack() as ctx:
        singles = ctx.enter_context(tc.tile_pool(name="singles", bufs=1))
        ident_f = singles.tile([P, P], F32)
        ident_h = singles.tile([P, P], F16)
        ones_h = singles.tile([HD, P], F16)
        temp_sb = singles.tile([HD, H], F32)
        bias_sb = singles.tile([P, CB], F32)
        s_sb = singles.tile([HD, 2 * H], F32)
        sumsq_sb = singles.tile([HD, 2 * H], F32)
        atsb = singles.tile([HD, H, P], F16)
        xt_sb = singles.tile([P, CB, N], F16)
        w2_sb = singles.tile([P, CB, W2PAD], F16)
        wvt_sb = singles.tile([HD, H, C], F16)
        wpe_sb = singles.tile([HD, H, C], F16)
        gsb = singles.tile([P, CB, C], F16)
        a2sb = singles.tile([P, CB, C2], F16)
        t1_sb = singles.tile([HD, H, C], F16)
        wbig_sb = singles.tile([P, CB, C], F16)

        make_identity(nc, ident_f)
        nc.vector.tensor_copy(out=ident_h, in_=ident_f)
        nc.vector.memset(ones_h, 1.0)
        nc.vector.memset(atsb, 0.0)
        nc.vector.memset(w2_sb[:, :, C2:W2PAD], 0.0)
        nc.scalar.dma_start(out=temp_sb, in_=temp[None, :].to_broadcast([HD, H]))
        nc.scalar.dma_start(out=bias_sb, in_=biasE[:, :])

        # ---------------- pass 1: G = x^T x (upper triangle) ----------------
        # x double-chunks alternate sync/scalar rings; weights and the x^T
        # prefetch are emitted after the loop so they queue behind the x
        # stream on their rings.
        gram_ctx = ExitStack()
        gram_pool = gram_ctx.enter_context(
            tc.tile_pool(name="gps", bufs=1, space="PSUM")
        )
        gtile = [
            gram_pool.tile([P, 512], F32, tag=f"g{i}", name=f"g{i}")
            for i in range(6)
        ]

        # first two chunks single-width so PE starts ~2us sooner
        chunk_plan = [(0, 1), (1, 1)] + [(2 * i, 2) for i in range(1, NCH2)]
        with tc.tile_pool(name="p1", bufs=12) as p1pool:
            for ci_, (h0_, nt) in enumerate(chunk_plan):
                xc = p1pool.tile([P, 2, C], F16, tag="xc")
                ring = nc.sync if ci_ % 2 == 0 else nc.scalar
                ring.dma_start(
                    out=xc[:, 0:nt, :],
                    in_=x[h0_ * P : (h0_ + nt) * P, :].rearrange(
                        "(two p) c -> p two c", p=P
                    ),
                )
                for t in range(nt):
                    first = ci_ == 0 and t == 0
                    last = ci_ == len(chunk_plan) - 1 and t == nt - 1
                    for r in range(CB):
                        lh = xc[:, t, r * P : (r + 1) * P]
                        for (bank, off, c0, ncols) in _G_RUNS[r]:
                            nc.tensor.matmul(
                                gtile[bank][:, off : off + ncols],
                                lhsT=lh,
                                rhs=xc[:, t, c0 * P : c0 * P + ncols],
                                start=(first and off == 0),
                                stop=last,
                                skip_group_check=True,
                            )

        nc.scalar.dma_start(
            out=w2_sb[:, :, 0:C2], in_=w2.rearrange("(cb p) j -> p cb j", p=P)
        )
        nc.scalar.dma_start(out=wvt_sb, in_=wvt[:, :, :])
        nc.scalar.dma_start(out=wpe_sb, in_=wpe[:, :, :])
        for n in range(8):
            nsl = slice(n * 512, (n + 1) * 512)
            nc.sync.dma_start(
                out=xt_sb[:, :, nsl],
                in_=xt[:, nsl].rearrange("(cb p) m -> p cb m", p=P),
            )

        # ---------------- finalize ----------------
        # G PSUM -> SBUF upper blocks, column-descending so A2 row 5 (which
        # needs no mirrored blocks) can start immediately; mirror transposes
        # run on PE underneath A2 row 5.
        for c in range(CB - 1, -1, -1):
            for r in range(0, c + 1):
                bank, off = _blk(_STARTS[r] + c - r)
                if (r + c) % 2 == 0:
                    nc.vector.tensor_copy(
                        out=gsb[:, r, c * P : (c + 1) * P],
                        in_=gtile[bank][:, off : off + P],
                    )
                else:
                    nc.scalar.copy(
                        out=gsb[:, r, c * P : (c + 1) * P],
                        in_=gtile[bank][:, off : off + P],
                    )
        gram_ctx.close()

        fs_ctx = ExitStack()
        at_ctx = ExitStack()
        atpool = at_ctx.enter_context(tc.tile_pool(name="atps", bufs=1, space="PSUM"))
        at1 = atpool.tile([P, 5 * HD], F32, tag="at1", name="at1")
        at2 = atpool.tile([P, 3 * HD], F32, tag="at2", name="at2")
        a2_ctx = ExitStack()
        a2ps = a2_ctx.enter_context(tc.tile_pool(name="a2ps", bufs=2, space="PSUM"))
        dscr = fs_ctx.enter_context(tc.tile_pool(name="dscr", bufs=1))

        def a2_row(r, j0):
            """A2[:, r, j0:j0+768] = sum_cb G[cb, r]^T @ w2[cb, j0:j0+768]"""
            a2p = a2ps.tile([P, 1024], F32, tag="a2p")
            for cb in range(CB):
                lh = gsb[:, cb, r * P : (r + 1) * P]
                nc.tensor.matmul(
                    a2p[:, 0:512], lhsT=lh, rhs=w2_sb[:, cb, j0 : j0 + 512],
                    start=(cb == 0), stop=(cb == CB - 1),
                    skip_group_check=True,
                )
                nc.tensor.matmul(
                    a2p[:, 512:768], lhsT=lh,
                    rhs=w2_sb[:, cb, j0 + 512 : j0 + 768],
                    start=(cb == 0), stop=(cb == CB - 1),
                    skip_group_check=True,
                )
            if r % 2 == 0:
                nc.vector.tensor_copy(
                    out=a2sb[:, r, j0 : j0 + C], in_=a2p[:, 0:C]
                )
            else:
                nc.scalar.copy(out=a2sb[:, r, j0 : j0 + C], in_=a2p[:, 0:C])

        # phase K row 5 first: all its G blocks are direct upper copies
        a2_row(5, C)

        # mirror the lower triangle of G via PE transposes (under A2 row 5)
        with tc.tile_pool(name="tpps", bufs=2, space="PSUM") as tppool:
            for r in range(CB - 2, -1, -1):
                for c in range(r + 1, CB):
                    tp = tppool.tile([P, P], F32, tag="tp")
                    nc.tensor.matmul(
                        tp,
                        lhsT=gsb[:, r, c * P : (c + 1) * P],
                        rhs=ident_h,
                        start=True,
                        stop=True,
                    )
                    nc.vector.tensor_copy(
                        out=gsb[:, c, r * P : (r + 1) * P], in_=tp
                    )

        kk_ctx = ExitStack()
        kkpool = kk_ctx.enter_context(tc.tile_pool(name="kkps", bufs=1, space="PSUM"))
        kk1 = kkpool.tile([P, 5 * HD], F32, tag="kk1", name="kk1")
        kk2 = kkpool.tile([P, 3 * HD], F32, tag="kk2", name="kk2")

        def grams_k(r, first, last):
            for h in range(H):
                b1, b2 = (at1, kk1) if h < 5 else (at2, kk2)
                co = HD * h if h < 5 else HD * (h - 5)
                rhs = a2sb[:, r, C + h * HD : C + (h + 1) * HD]
                nc.tensor.matmul(
                    b1[:, co : co + HD],
                    lhsT=w2_sb[:, r, h * HD : h * HD + P],
                    rhs=rhs,
                    start=(first and h in (0, 5)), stop=last,
                    skip_group_check=True,
                )
                nc.tensor.matmul(
                    b2[:, co : co + HD],
                    lhsT=w2_sb[:, r, C + h * HD : C + h * HD + P],
                    rhs=rhs,
                    start=(first and h in (0, 5)), stop=last,
                    skip_group_check=True,
                )

        rows = [4, 3, 2, 1, 0]
        a2_row(4, C)
        grams_k(5, True, False)
        for idx, r in enumerate(rows[1:]):
            a2_row(r, C)
            grams_k(rows[idx], False, False)
        grams_k(0, False, True)

        def diag_extract(b1, b2, out5, out3):
            d1 = dscr.tile([HD, 5, HD], F32, tag="dg1")
            d2 = dscr.tile([HD, 3, HD], F32, tag="dg2")
            nc.vector.tensor_tensor(
                out=d1,
                in0=b1[0:HD, :].rearrange("p (h e) -> p h e", e=HD),
                in1=ident_h[0:HD, None, 0:HD].to_broadcast([HD, 5, HD]),
                op=mybir.AluOpType.mult,
            )
            nc.vector.tensor_tensor(
                out=d2,
                in0=b2[0:HD, :].rearrange("p (h e) -> p h e", e=HD),
                in1=ident_h[0:HD, None, 0:HD].to_broadcast([HD, 3, HD]),
                op=mybir.AluOpType.mult,
            )
            nc.vector.tensor_reduce(
                out=out5, in_=d1, axis=mybir.AxisListType.X,
                op=mybir.AluOpType.add,
            )
            nc.vector.tensor_reduce(
                out=out3, in_=d2, axis=mybir.AxisListType.X,
                op=mybir.AluOpType.add,
            )

        # ---- sumsq_k extraction + s_k chain (overlaps phase Q's PE work) --
        diag_extract(kk1, kk2, sumsq_sb[:, H : H + 5], sumsq_sb[:, H + 5 : 2 * H])
        kk_ctx.close()
        nc.scalar.sqrt(out=s_sb[:, H : 2 * H], in_=sumsq_sb[:, H : 2 * H])
        nc.vector.reciprocal(out=s_sb[:, H : 2 * H], in_=s_sb[:, H : 2 * H])
        diag_all = dscr.tile([HD, H, HD], F16, tag="diag_all")
        nc.vector.tensor_tensor(
            out=diag_all,
            in0=ident_h[0:HD, None, 0:HD].to_broadcast([HD, H, HD]),
            in1=s_sb[:, H : 2 * H, None].to_broadcast([HD, H, HD]),
            op=mybir.AluOpType.mult,
        )
        # temperature folded in here, off the critical path
        nc.vector.tensor_tensor(
            out=diag_all,
            in0=diag_all,
            in1=temp_sb[:, :, None].to_broadcast([HD, H, HD]),
            op=mybir.AluOpType.mult,
        )

        # ---- phase Q (rows descending) ----
        qq_ctx = ExitStack()
        qqpool = qq_ctx.enter_context(tc.tile_pool(name="qqps", bufs=1, space="PSUM"))
        qq1 = qqpool.tile([P, 5 * HD], F32, tag="qq1", name="qq1")
        qq2 = qqpool.tile([P, 3 * HD], F32, tag="qq2", name="qq2")

        def grams_q(r, first, last):
            for h in range(H):
                b1 = qq1 if h < 5 else qq2
                co = HD * h if h < 5 else HD * (h - 5)
                nc.tensor.matmul(
                    b1[:, co : co + HD],
                    lhsT=w2_sb[:, r, h * HD : h * HD + P],
                    rhs=a2sb[:, r, h * HD : (h + 1) * HD],
                    start=(first and h in (0, 5)), stop=last,
                    skip_group_check=True,
                )

        a2_row(5, 0)
        a2_row(4, 0)
        grams_q(5, True, False)
        for idx, r in enumerate(rows[1:]):
            a2_row(r, 0)
            grams_q(rows[idx], False, False)
        grams_q(0, False, True)

        diag_extract(qq1, qq2, sumsq_sb[:, 0:5], sumsq_sb[:, 5:H])
        qq_ctx.close()
        a2_ctx.close()
        nc.scalar.sqrt(out=s_sb[:, 0:H], in_=sumsq_sb[:, 0:H])
        nc.vector.reciprocal(out=s_sb[:, 0:H], in_=s_sb[:, 0:H])

        # combined scale [d,h,e] = s_q[d,h] * s_k[e,h] via ones^T @ diag_all
        skrep = dscr.tile([HD, H, HD], F32, tag="skrep")
        with tc.tile_pool(name="skps", bufs=1, space="PSUM") as skpool:
            skp = skpool.tile([P, 1024], F32, tag="skp")
            df = diag_all.rearrange("p h e -> p (h e)")
            nc.tensor.matmul(
                skp[:, 0:512], lhsT=ones_h, rhs=df[:, 0:512],
                start=True, stop=True,
            )
            nc.tensor.matmul(
                skp[:, 512:768], lhsT=ones_h, rhs=df[:, 512:768],
                start=True, stop=True,
            )
            nc.vector.tensor_copy(
                out=skrep.rearrange("p h e -> p (h e)"), in_=skp[0:HD, 0:768]
            )
            nc.vector.tensor_tensor(
                out=skrep, in0=skrep,
                in1=s_sb[:, 0:H, None].to_broadcast([HD, H, HD]),
                op=mybir.AluOpType.mult,
            )

            # softmax per head-group (no max subtraction: |logit| <= temp)
            # T1_h = attn_h^T @ Wproj_h follows each group on PE
            ga = dscr.tile([HD, H, HD], F32, tag="ga")
            with tc.tile_pool(name="t1ps", bufs=2, space="PSUM") as t1ps:
                for g, (h0, nh) in enumerate(((0, 5), (5, 3))):
                    bank = at1 if g == 0 else at2
                    gag = ga[:, h0 : h0 + nh, :]
                    nc.vector.tensor_tensor(
                        out=gag,
                        in0=bank[0:HD, :].rearrange("p (h e) -> p h e", e=HD),
                        in1=skrep[:, h0 : h0 + nh, :],
                        op=mybir.AluOpType.mult,
                    )
                    nc.scalar.activation(
                        out=gag, in_=gag,
                        func=mybir.ActivationFunctionType.Exp,
                        bias=0.0, scale=1.0,
                    )
                    rsum = dscr.tile([HD, H], F32, tag=f"rsum{g}")
                    nc.vector.tensor_reduce(
                        out=rsum[:, 0:nh], in_=gag, axis=mybir.AxisListType.X,
                        op=mybir.AluOpType.add,
                    )
                    nc.vector.reciprocal(out=rsum[:, 0:nh], in_=rsum[:, 0:nh])
                    nc.vector.tensor_tensor(
                        out=atsb[:, h0 : h0 + nh, 0:HD], in0=gag,
                        in1=rsum[:, 0:nh, None].to_broadcast([HD, nh, HD]),
                        op=mybir.AluOpType.mult,
                    )
                    for h in range(h0, h0 + nh):
                        t1p = t1ps.tile([P, 1024], F32, tag="t1p")
                        lh = atsb[:, h, :]
                        nc.tensor.matmul(
                            t1p[:, 0:512], lhsT=lh, rhs=wpe_sb[:, h, 0:512],
                            start=True, stop=True,
                        )
                        nc.tensor.matmul(
                            t1p[:, 512:768], lhsT=lh, rhs=wpe_sb[:, h, 512:C],
                            start=True, stop=True,
                        )
                        if h % 2 == 0:
                            nc.vector.tensor_copy(
                                out=t1_sb[:, h, :], in_=t1p[0:HD, 0:C]
                            )
                        else:
                            nc.scalar.copy(
                                out=t1_sb[:, h, :], in_=t1p[0:HD, 0:C]
                            )
        at_ctx.close()

        # Wbig = sum_h Wv_h @ T1_h
        with tc.tile_pool(name="wbps", bufs=2, space="PSUM") as wbps:
            for m in range(CB):
                wbp = wbps.tile([P, 1024], F32, tag="wbp")
                for h in range(H):
                    lh = wvt_sb[:, h, m * P : (m + 1) * P]
                    nc.tensor.matmul(
                        wbp[:, 0:512], lhsT=lh, rhs=t1_sb[:, h, 0:512],
                        start=(h == 0), stop=(h == H - 1),
                        skip_group_check=True,
                    )
                    nc.tensor.matmul(
                        wbp[:, 512:768], lhsT=lh, rhs=t1_sb[:, h, 512:C],
                        start=(h == 0), stop=(h == H - 1),
                        skip_group_check=True,
                    )
                if m % 2 == 0:
                    nc.vector.tensor_copy(out=wbig_sb[:, m, :], in_=wbp[:, 0:C])
                else:
                    nc.scalar.copy(out=wbig_sb[:, m, :], in_=wbp[:, 0:C])
        fs_ctx.close()

        # ---------------- pass 2: y^T = Wbig^T @ x^T + b (fp16) -------------
        with tc.tile_pool(name="yps", bufs=8, space="PSUM") as yps, \
             tc.tile_pool(name="ysbp", bufs=4) as ypool:
            for n in range(8):
                nsl = slice(n * 512, (n + 1) * 512)
                for co in range(CB):
                    ypb = yps.tile([P, 512], F32, tag="ypb")
                    for ci in range(CB):
                        nc.tensor.matmul(
                            ypb,
                            lhsT=wbig_sb[:, ci, co * P : (co + 1) * P],
                            rhs=xt_sb[:, ci, nsl],
                            start=(ci == 0),
                            stop=(ci == CB - 1),
                        )
                    ysb = ypool.tile([P, 512], F16, tag="ysb")
                    nc.vector.tensor_tensor(
                        out=ysb,
                        in0=ypb,
                        in1=bias_sb[:, co : co + 1].to_broadcast([P, 512]),
                        op=mybir.AluOpType.add,
                    )
                    ring = nc.sync if co % 2 == 0 else nc.scalar
                    ring.dma_start(out=yt[co * P : (co + 1) * P, nsl], in_=ysb)

    nc.compile()
    return nc


def prep_inputs(x, Wqkv, temperature, Wproj, bproj):
    B = x.shape[0]
    wqkv = np.asarray(Wqkv, dtype=np.float32)
    w2 = np.ascontiguousarray(wqkv[:, :C2].astype(np.float16))
    wvt = np.ascontiguousarray(
        wqkv[:, C2:].T.reshape(H, HD, C).transpose(1, 0, 2).astype(np.float16)
    )
    wpe = np.ascontiguousarray(
        np.asarray(Wproj, dtype=np.float32)
        .reshape(H, HD, C)
        .transpose(1, 0, 2)
        .astype(np.float16)
    )
    temp = np.ascontiguousarray(np.asarray(temperature, dtype=np.float32).reshape(H))
    biasE = np.ascontiguousarray(
        np.asarray(bproj, dtype=np.float32).reshape(CB, P).T
    )
    xf = np.asarray(x, dtype=np.float32)
    in_maps = [
        {
            "x": np.ascontiguousarray(xf[b].astype(np.float16)),
            "xt": np.ascontiguousarray(xf[b].T.astype(np.float16)),
            "w2": w2,
            "wvt": wvt,
            "wpe": wpe,
            "temp": temp,
            "biasE": biasE,
        }
        for b in range(B)
    ]
    return in_maps


def kernel(x, Wqkv, temperature, Wproj, bproj):
    from concourse.bass_utils import run_bass_kernel_spmd

    B = x.shape[0]
    key = "nc"
    if key not in _CACHE:
        _CACHE[key] = _build()
    nc = _CACHE[key]

    in_maps = prep_inputs(x, Wqkv, temperature, Wproj, bproj)
    res = run_bass_kernel_spmd(nc, in_maps, core_ids=list(range(B)))
    out = np.stack(
        [res.results[b]["yt"].T.astype(np.float32) for b in range(B)], axis=0
    )
    return np.ascontiguousarray(out)


if __name__ == "__main__":
    rng = np.random.default_rng(0)
    inputs = {
        "x": rng.standard_normal((8, N, C), dtype=np.float32),
        "Wqkv": (rng.standard_normal((C, NC3)) / np.sqrt(C)).astype(np.float32),
        "temperature": np.ones((H, 1, 1), dtype=np.float32),
        "Wproj": (rng.standard_normal((C, C)) / np.sqrt(C)).astype(np.float32),
        "bproj": (rng.standard_normal(C) * 0.01).astype(np.float32),
    }
    out = kernel(**inputs)
    print(out.shape, out.dtype)
